# revision 4
# baseline (speedup 1.0000x reference)
"""Trainium2 Bass kernel for nn_Encoder_80616536146562 (graph-LSTM encoder).

Reference computation (B=4, T=12, N=4096, F=16):
  per step t:
    gx = relu(adj @ (x_t @ Wx) + bx); gh = relu(adj @ (h @ Wh) + bh)
    LSTM gates -> c, h2
    sh = relu(adj @ (h2 @ Wsh) + bsh); sm = relu(adj @ (m @ Wsm) + bsm)
    memory gates -> m, h = m * o2
  outputs: hidden_states [B,T,N,F], last_h, last_c, last_m

Strategy: 8-way row-shard of adj (512 rows/core) kept resident in SBUF as
fp16 (scaled by 4096 to avoid fp16 subnormals; un-scaled when leaving PSUM).
Associativity: compute Y = adj @ z first (F cols per state), then tiny Y @ W
matmuls on-chip (PE transpose + W-matmuls). adj@x_t is precomputed for all t
upfront. Node states h/m/h2 are all-gathered in fp16 across the 8 cores twice
per step via collective_compute through DRAM bounce buffers.
"""
import sys

if "/opt/trn_rl_repo" not in sys.path:
    sys.path.insert(0, "/opt/trn_rl_repo")

import numpy as np

B, T, N, F = 4, 12, 4096, 16
NCORES = 8
RPC = N // NCORES          # rows per core = 512
NM = RPC // 128            # m-tiles per core = 4
NK = N // 128              # k-tiles = 32
G1 = 4 * F                 # 64  (f|i|c|o gates)
G2 = 3 * F                 # 48  (i|g|o gates)
BF = B * F                 # 64
SCALE = 4096.0

_cache = {}


def _build_nc():
    import concourse.bass as bass
    import concourse.mybir as mybir
    import concourse.tile as tile

    f32, f16 = mybir.dt.float32, mybir.dt.float16
    AF = mybir.ActivationFunctionType
    OP = mybir.AluOpType

    nc = bass.Bass(trn_type="TRN2", num_devices=NCORES)

    # ---- per-core inputs ----
    adjT = nc.dram_tensor("adjT", [N, RPC], f32, kind="ExternalInput")
    xr = nc.dram_tensor("xr", [N, T * BF], f32, kind="ExternalInput")   # x[n, (t,b,f)]
    wx = nc.dram_tensor("wx", [BF, B * G1], f32, kind="ExternalInput")
    wh = nc.dram_tensor("wh", [BF, B * G1], f32, kind="ExternalInput")
    wsh = nc.dram_tensor("wsh", [BF, B * G2], f32, kind="ExternalInput")
    wsm = nc.dram_tensor("wsm", [BF, B * G2], f32, kind="ExternalInput")
    bxt = nc.dram_tensor("bxt", [128, B * G1], f32, kind="ExternalInput")
    bht = nc.dram_tensor("bht", [128, B * G1], f32, kind="ExternalInput")
    bsht = nc.dram_tensor("bsht", [128, B * G2], f32, kind="ExternalInput")
    bsmt = nc.dram_tensor("bsmt", [128, B * G2], f32, kind="ExternalInput")
    ident = nc.dram_tensor("ident", [128, 128], f32, kind="ExternalInput")
    # ---- per-core outputs ----
    hs = nc.dram_tensor("hs", [T, NM, B, 128, F], f32, kind="ExternalOutput")
    lc = nc.dram_tensor("lc", [NM, B, 128, F], f32, kind="ExternalOutput")
    lm = nc.dram_tensor("lm", [NM, B, 128, F], f32, kind="ExternalOutput")

    with tile.TileContext(nc) as tc:
        with tc.tile_pool(name="const", bufs=1) as constp, \
             tc.tile_pool(name="stage", bufs=2) as stagep, \
             tc.tile_pool(name="state", bufs=1) as statep, \
             tc.tile_pool(name="dram", bufs=2, space="DRAM") as dramp, \
             tc.tile_pool(name="work", bufs=2) as workp, \
             tc.tile_pool(name="t1p", bufs=4) as t1p:

            # ===== constants / weights =====
            adj16 = constp.tile([128, NK * RPC], f16)        # 4 MB: lhsT tiles
            for k in range(NK):
                a32 = stagep.tile([128, RPC], f32, tag="a32")
                nc.sync.dma_start(a32[:], adjT[k * 128:(k + 1) * 128, :])
                nc.vector.tensor_scalar_mul(
                    adj16[:, k * RPC:(k + 1) * RPC], a32[:], SCALE)

            id16 = constp.tile([128, 128], f16)
            i32 = stagep.tile([128, 128], f32, tag="a32")
            nc.sync.dma_start(i32[:], ident[:])
            nc.vector.tensor_copy(id16[:], i32[:])

            w16 = {}
            for name, t_, gw in (("wx", wx, B * G1), ("wh", wh, B * G1),
                                 ("wsh", wsh, B * G2), ("wsm", wsm, B * G2)):
                ww = stagep.tile([BF, gw], f32, tag="w32", name=f"w32_{name}")
                nc.sync.dma_start(ww[:], t_[:])
                w1 = constp.tile([BF, gw], f16, name=f"w16_{name}")
                nc.vector.tensor_copy(w1[:], ww[:])
                w16[name] = w1

            bias = {}
            for name, t_, gw in (("bxt", bxt, B * G1), ("bht", bht, B * G1),
                                 ("bsht", bsht, B * G2), ("bsmt", bsmt, B * G2)):
                bb = constp.tile([128, gw], f32, name=f"b_{name}")
                nc.sync.dma_start(bb[:], t_[:])
                bias[name] = bb

            # ===== x-pass: Ax[m] = (adj @ x)  for all t, fp16, un-scaled =====
            Ax = [constp.tile([128, T * BF], f16, name=f"Ax{m}") for m in range(NM)]
            HALF = T * BF // 2                                # 384 cols per psum
            with tc.tile_pool(name="xps", bufs=1, space="PSUM") as xps:
                ax_ps = [[xps.tile([128, HALF], f32, name=f"axps{m}_{h}")
                          for h in range(2)] for m in range(NM)]
                for k in range(NK):
                    xs32 = stagep.tile([128, T * BF], f32, tag="xs32")
                    nc.sync.dma_start(xs32[:], xr[k * 128:(k + 1) * 128, :])
                    x16k = stagep.tile([128, T * BF], f16, tag="x16k")
                    nc.vector.tensor_copy(x16k[:], xs32[:])
                    for m in range(NM):
                        for h in range(2):
                            nc.tensor.matmul(
                                ax_ps[m][h][:],
                                adj16[:, k * RPC + m * 128:k * RPC + (m + 1) * 128],
                                x16k[:, h * HALF:(h + 1) * HALF],
                                start=(k == 0), stop=(k == NK - 1))
                for m in range(NM):
                    for h in range(2):
                        nc.vector.tensor_scalar_mul(
                            Ax[m][:, h * HALF:(h + 1) * HALF],
                            ax_ps[m][h][:], 1.0 / SCALE)

            # AxT[(b,f), (t, m, node)] fp16 — transposed x-conv activations
            AxT = constp.tile([BF, T * NM * 128], f16)
            with tc.tile_pool(name="tps", bufs=4, space="PSUM") as tps:
                for t in range(T):
                    for m in range(NM):
                        tp = tps.tile([BF, 128], f16, tag="tp")
                        nc.tensor.transpose(
                            tp[:], Ax[m][:, t * BF:(t + 1) * BF], id16[:])
                        nc.vector.tensor_copy(
                            AxT[:, (t * NM + m) * 128:(t * NM + m + 1) * 128], tp[:])

            # ===== recurrent state =====
            hm16 = statep.tile([128, NK * 2 * BF], f16)   # gathered [h|m] per k-tile
            h2g = statep.tile([128, NK * BF], f16)        # gathered h2 per k-tile
            c32 = statep.tile([128, NM * BF], f32)
            m32 = statep.tile([128, NM * BF], f32)
            nc.gpsimd.memset(hm16[:], 1.0)
            nc.gpsimd.memset(c32[:], 1.0)
            nc.gpsimd.memset(m32[:], 1.0)

            with tc.tile_pool(name="ps128", bufs=2, space="PSUM") as ps128, \
                 tc.tile_pool(name="gxh", bufs=2, space="PSUM") as gxhp, \
                 tc.tile_pool(name="ps2", bufs=2, space="PSUM") as ps2, \
                 tc.tile_pool(name="shm", bufs=2, space="PSUM") as shmp:

                for t in range(T):
                    t1s = []
                    hmn16 = workp.tile([128, NM * 2 * BF], f16, tag="hmn16")
                    h2_16 = workp.tile([128, NM * BF], f16, tag="h2_16")
                    hnew32 = workp.tile([128, NM * BF], f32, tag="hnew32")

                    # ---------- stage 1 ----------
                    for m in range(NM):
                        y1 = ps128.tile([128, 2 * BF], f32, tag="y1")
                        for k in range(NK):
                            nc.tensor.matmul(
                                y1[:],
                                adj16[:, k * RPC + m * 128:k * RPC + (m + 1) * 128],
                                hm16[:, k * 2 * BF:(k + 1) * 2 * BF],
                                start=(k == 0), stop=(k == NK - 1))
                        y1s = workp.tile([128, 2 * BF], f16, tag="y1s")
                        nc.vector.tensor_scalar_mul(y1s[:], y1[:], 1.0 / SCALE)
                        t1h_ps = ps128.tile([BF, 128], f16, tag="y1", name="t1hps")
                        nc.tensor.transpose(t1h_ps[:], y1s[:, 0:BF], id16[:])
                        t1h = workp.tile([BF, 128], f16, tag="t1h")
                        nc.vector.tensor_copy(t1h[:], t1h_ps[:])
                        t1m_ps = ps128.tile([BF, 128], f16, tag="y1", name="t1mps")
                        nc.tensor.transpose(t1m_ps[:], y1s[:, BF:2 * BF], id16[:])
                        t1m = t1p.tile([BF, 128], f16, tag="t1m")
                        nc.vector.tensor_copy(t1m[:], t1m_ps[:])
                        t1s.append(t1m)

                        gx = gxhp.tile([128, B * G1], f32, tag="g")
                        gh = gxhp.tile([128, B * G1], f32, tag="g", name="gh")
                        nc.tensor.matmul(
                            gx[:],
                            AxT[:, (t * NM + m) * 128:(t * NM + m + 1) * 128],
                            w16["wx"][:], start=True, stop=True)
                        nc.tensor.matmul(
                            gh[:], t1h[:], w16["wh"][:], start=True, stop=True)
                        rgx = workp.tile([128, B * G1], f32, tag="rgx")
                        nc.vector.tensor_tensor(rgx[:], gx[:], bias["bxt"][:], OP.add)
                        nc.scalar.activation(rgx[:], rgx[:], AF.Relu)
                        rgh = workp.tile([128, B * G1], f32, tag="rgh")
                        nc.vector.tensor_tensor(rgh[:], gh[:], bias["bht"][:], OP.add)
                        nc.scalar.activation(rgh[:], rgh[:], AF.Relu)
                        s = workp.tile([128, B, G1], f32, tag="s")
                        nc.vector.tensor_tensor(
                            s[:], rgx[:].rearrange("p (b g) -> p b g", b=B),
                            rgh[:].rearrange("p (b g) -> p b g", b=B), OP.add)
                        # sigmoid(f,i) ; tanh(c) ; sigmoid(o)
                        nc.scalar.activation(s[:, :, 0:2 * F], s[:, :, 0:2 * F], AF.Sigmoid)
                        nc.scalar.activation(s[:, :, 3 * F:4 * F], s[:, :, 3 * F:4 * F], AF.Sigmoid)
                        nc.scalar.activation(s[:, :, 2 * F:3 * F], s[:, :, 2 * F:3 * F], AF.Tanh)
                        csl = c32[:, m * BF:(m + 1) * BF].rearrange(
                            "p (b f) -> p b f", b=B)
                        tmp1 = workp.tile([128, B, F], f32, tag="tmp1")
                        tmp2 = workp.tile([128, B, F], f32, tag="tmp2")
                        nc.vector.tensor_tensor(tmp1[:], s[:, :, 0:F], csl, OP.mult)
                        nc.vector.tensor_tensor(tmp2[:], s[:, :, F:2 * F],
                                                s[:, :, 2 * F:3 * F], OP.mult)
                        nc.vector.tensor_tensor(csl, tmp1[:], tmp2[:], OP.add)
                        th = workp.tile([128, B, F], f32, tag="th")
                        nc.scalar.activation(th[:], csl, AF.Tanh)
                        nc.vector.tensor_tensor(
                            h2_16[:, m * BF:(m + 1) * BF].rearrange(
                                "p (b f) -> p b f", b=B),
                            s[:, :, 3 * F:4 * F], th[:], OP.mult)

                    # ---------- all-gather h2 ----------
                    agA_in = dramp.tile([RPC, BF], f16, tag="agA_in")
                    nc.sync.dma_start(
                        agA_in.rearrange("(m p) f -> p m f", p=128), h2_16[:])
                    agA_out = dramp.tile([N, BF], f16, tag="agA_out",
                                         addr_space="Shared")
                    nc.gpsimd.collective_compute(
                        "AllGather", OP.bypass,
                        replica_groups=[list(range(NCORES))],
                        ins=[agA_in[:]], outs=[agA_out[:]])
                    for half in range(2):
                        nc.sync.dma_start(
                            h2g[:, half * NK // 2 * BF:(half + 1) * NK // 2 * BF]
                            .rearrange("p (k f) -> p k f", k=NK // 2),
                            agA_out.rearrange("(k p) f -> p k f", p=128)
                            [:, half * NK // 2:(half + 1) * NK // 2, :])

                    # ---------- stage 2 ----------
                    for m in range(NM):
                        y2 = ps2.tile([128, BF], f32, tag="y2")
                        for k in range(NK):
                            nc.tensor.matmul(
                                y2[:],
                                adj16[:, k * RPC + m * 128:k * RPC + (m + 1) * 128],
                                h2g[:, k * BF:(k + 1) * BF],
                                start=(k == 0), stop=(k == NK - 1))
                        y2s = workp.tile([128, BF], f16, tag="y2s")
                        nc.vector.tensor_scalar_mul(y2s[:], y2[:], 1.0 / SCALE)
                        t2_ps = ps2.tile([BF, 128], f16, tag="y2", name="t2ps")
                        nc.tensor.transpose(t2_ps[:], y2s[:], id16[:])
                        t2 = workp.tile([BF, 128], f16, tag="t2")
                        nc.vector.tensor_copy(t2[:], t2_ps[:])

                        sh = shmp.tile([128, B * G2], f32, tag="sg")
                        sm = shmp.tile([128, B * G2], f32, tag="sg", name="sm")
                        nc.tensor.matmul(
                            sh[:], t2[:], w16["wsh"][:], start=True, stop=True)
                        nc.tensor.matmul(
                            sm[:], t1s[m][:], w16["wsm"][:], start=True, stop=True)
                        rsh = workp.tile([128, B * G2], f32, tag="rsh")
                        nc.vector.tensor_tensor(rsh[:], sh[:], bias["bsht"][:], OP.add)
                        nc.scalar.activation(rsh[:], rsh[:], AF.Relu)
                        rsm = workp.tile([128, B * G2], f32, tag="rsm")
                        nc.vector.tensor_tensor(rsm[:], sm[:], bias["bsmt"][:], OP.add)
                        nc.scalar.activation(rsm[:], rsm[:], AF.Relu)
                        s2 = workp.tile([128, B, G2], f32, tag="s2")
                        nc.vector.tensor_tensor(
                            s2[:], rsh[:].rearrange("p (b g) -> p b g", b=B),
                            rsm[:].rearrange("p (b g) -> p b g", b=B), OP.add)
                        nc.scalar.activation(s2[:], s2[:], AF.Sigmoid)

                        msl = m32[:, m * BF:(m + 1) * BF].rearrange(
                            "p (b f) -> p b f", b=B)
                        tmp1 = workp.tile([128, B, F], f32, tag="tmp1")
                        tmp2 = workp.tile([128, B, F], f32, tag="tmp2")
                        # m = i2*m + (1-i2)*g2 ; h = m*o2
                        nc.vector.tensor_tensor(tmp1[:], s2[:, :, 0:F], msl, OP.mult)
                        nc.vector.tensor_tensor(tmp2[:], s2[:, :, 0:F],
                                                s2[:, :, F:2 * F], OP.mult)
                        nc.vector.tensor_tensor(tmp2[:], s2[:, :, F:2 * F],
                                                tmp2[:], OP.subtract)
                        nc.vector.tensor_tensor(msl, tmp1[:], tmp2[:], OP.add)
                        hsl = hnew32[:, m * BF:(m + 1) * BF].rearrange(
                            "p (b f) -> p b f", b=B)
                        nc.vector.tensor_tensor(hsl, msl, s2[:, :, 2 * F:3 * F], OP.mult)
                        # fp16 copies for the next gather
                        nc.vector.tensor_copy(
                            hmn16[:, m * 2 * BF:m * 2 * BF + BF],
                            hnew32[:, m * BF:(m + 1) * BF])
                        nc.vector.tensor_copy(
                            hmn16[:, m * 2 * BF + BF:(m + 1) * 2 * BF],
                            m32[:, m * BF:(m + 1) * BF])

                    # ---------- outputs + all-gather [h|m] ----------
                    nc.sync.dma_start(
                        hs[t].rearrange("m b p f -> p m b f"),
                        hnew32[:].rearrange("p (m b f) -> p m b f", m=NM, b=B))

                    if t < T - 1:
                        agB_in = dramp.tile([RPC, 2 * BF], f16, tag="agB_in")
                        nc.sync.dma_start(
                            agB_in.rearrange("(m p) c -> p m c", p=128), hmn16[:])
                        agB_out = dramp.tile([N, 2 * BF], f16, tag="agB_out",
                                             addr_space="Shared")
                        nc.gpsimd.collective_compute(
                            "AllGather", OP.bypass,
                            replica_groups=[list(range(NCORES))],
                            ins=[agB_in[:]], outs=[agB_out[:]])
                        for half in range(2):
                            nc.sync.dma_start(
                                hm16[:, half * NK * BF:(half + 1) * NK * BF]
                                .rearrange("p (k c) -> p k c", k=NK // 2),
                                agB_out.rearrange("(k p) c -> p k c", p=128)
                                [:, half * NK // 2:(half + 1) * NK // 2, :])

                nc.sync.dma_start(
                    lc[:].rearrange("m b p f -> p m b f"),
                    c32[:].rearrange("p (m b f) -> p m b f", m=NM, b=B))
                nc.sync.dma_start(
                    lm[:].rearrange("m b p f -> p m b f"),
                    m32[:].rearrange("p (m b f) -> p m b f", m=NM, b=B))

    _legalize_waits(nc)
    return nc


def _legalize_waits(nc):
    """Walrus accepts at most 1 sync-wait per instruction (2 for
    EventSemaphore). Move excess waits onto standalone EventSemaphore
    instructions on the same engine, inserted just before."""
    import concourse.mybir as mybir

    n_split = 0
    for fn in nc.m.functions:
        for bb in fn.blocks:
            newl = []
            changed = False
            for ins in bb.instructions:
                si = ins.sync_info
                waits = list(si.on_wait) if (si is not None and si.on_wait) else []
                cap = 2 if isinstance(ins, mybir.InstEventSemaphore) else 1
                if len(waits) > cap:
                    extra, keep = waits[:-cap], waits[-cap:]
                    for i in range(0, len(extra), 2):
                        ev = mybir.InstEventSemaphore(
                            name=f"{ins.name}_xw{i}",
                            engine=ins.engine,
                            sync_info=mybir.SyncInfo(
                                on_wait=list(extra[i:i + 2]), on_update=[]),
                        )
                        newl.append(ev)
                        n_split += 1
                    ins.sync_info = mybir.SyncInfo(
                        on_wait=list(keep), on_update=list(si.on_update))
                    changed = True
                newl.append(ins)
            if changed:
                bb.instructions = newl
    return n_split


def run(inputs, trace=False):
    from concourse.bass_utils import run_bass_kernel_spmd

    if "nc" not in _cache:
        _cache["nc"] = _build_nc()
    nc = _cache["nc"]

    x = np.ascontiguousarray(inputs["x"], dtype=np.float32)
    adj = np.ascontiguousarray(inputs["adj"], dtype=np.float32)
    # x rearranged to [n, (t, b, f)]
    xr = np.ascontiguousarray(x.transpose(2, 1, 0, 3)).reshape(N, T * B * F)
    ident = np.eye(128, dtype=np.float32)

    def btile(bvec, gw):
        return np.ascontiguousarray(
            np.broadcast_to(np.tile(np.asarray(bvec, np.float32), B), (128, B * gw)))

    def bdiag(w):
        w = np.asarray(w, np.float32)
        f, g = w.shape
        out = np.zeros((B * f, B * g), np.float32)
        for b in range(B):
            out[b * f:(b + 1) * f, b * g:(b + 1) * g] = w
        return out

    common = {
        "xr": xr,
        "wx": bdiag(inputs["Wx"]),
        "wh": bdiag(inputs["Wh"]),
        "wsh": bdiag(inputs["Wsh"]),
        "wsm": bdiag(inputs["Wsm"]),
        "bxt": btile(inputs["bx"], G1),
        "bht": btile(inputs["bh"], G1),
        "bsht": btile(inputs["bsh"], G2),
        "bsmt": btile(inputs["bsm"], G2),
        "ident": ident,
    }
    in_maps = []
    for c in range(NCORES):
        m = dict(common)
        m["adjT"] = np.ascontiguousarray(adj[c * RPC:(c + 1) * RPC, :].T)
        in_maps.append(m)

    res = run_bass_kernel_spmd(
        nc, in_maps, core_ids=list(range(NCORES)), trace=trace)

    hs_parts, lc_parts, lm_parts = [], [], []
    for c in range(NCORES):
        r = res.results[c]
        # hs [T, NM, B, 128, F] -> [B, T, RPC, F]
        hs_parts.append(r["hs"].transpose(2, 0, 1, 3, 4).reshape(B, T, RPC, F))
        lc_parts.append(r["lc"].transpose(1, 0, 2, 3).reshape(B, RPC, F))
        lm_parts.append(r["lm"].transpose(1, 0, 2, 3).reshape(B, RPC, F))
    hidden = np.concatenate(hs_parts, axis=2)
    last_c = np.concatenate(lc_parts, axis=1)
    last_m = np.concatenate(lm_parts, axis=1)
    last_h = np.ascontiguousarray(hidden[:, T - 1])
    return (hidden, last_h, last_c, last_m), res


def kernel(**inputs):
    out, _ = run(inputs, trace=False)
    return out
